# revision 50
# baseline (speedup 1.0000x reference)
"""Multi-head attention (2D-RoPE, masked softmax) on 8 Trainium2 NeuronCores.

Sharding: 4 head-groups (3 heads each) x 2 query-halves (1160 rows each).
Each core computes full attention for its 3 heads over its 1160 query rows
against all 2320 keys, plus its share of the output projection; the host
sums the 8 partial projections and adds the (folded) biases.

v2 changes vs the first working kernel (164.6us):
  - DMA head: inputs arrive in 15 consolidated DMAs (priority-ordered,
    rearranged-on-read) instead of ~52; cq/sq tables dropped entirely
    (the permuted ck/sk's first 1160 columns ARE the query tables);
    Wproj and the output partials are bf16.  PE warm-up matmuls +
    an exp-table-load dummy run at t=0 so the HAM clock is at 2.4GHz
    and the ACT tables are resident when real work arrives.
  - Scores: K=64 row-tiled matmul PAIRS (tile_position (0,0)/(64,0) via
    operand base partitions) run two key-chunks concurrently on the two
    64-row halves of the PE array -- needs K^T and Q duplicated on
    partitions 64:128 (done with two extra [64,n] adds in the rope).
  - QKV: head2's K-rows and Q-rows are stacked into one M=128 stationary
    (G1) so its projection streams the sequence once, not twice.
  - exp in chunk-TRIPLES ([128,3,512] psum groups) -> 21 instead of 30
    ACTIVATE instructions per head.
  - proj: heads 0,1 ctx stacked into one [128, SQ] buffer -> K=128
    contraction (2 matmuls per half-slice instead of 3); their 1/Z
    broadcasts share one psum via col-tiled ones-matmuls.
"""
import sys
if '/opt/trn_rl_repo' not in sys.path:
    sys.path.insert(0, '/opt/trn_rl_repo')
import numpy as np

SEQ, E, NH, D = 2320, 768, 12, 64
GRID, TASK = 48, 16
SQ = SEQ // 2           # query rows per core
HG = 3                  # heads per core
SCALE = D ** -0.5
EC = 6                  # embed chunks of 128
L_TILES = [(0, 512), (512, 392), (904, 256)]
N_TILES = [(0, 512), (512, 512), (1024, 512), (1536, 512), (2048, 272)]
MC = [(i * 128, min(128, SEQ - i * 128)) for i in range(19)]
PT = [(i * 128, min(128, SQ - i * 128)) for i in range(10)]
GROUPS = [tuple(range(g, min(g + 2, 19))) for g in range(0, 19, 2)]
XP = 8                  # xt column pieces
XPW = SEQ // XP         # 290

_prog = None


def _build(scores_tiled=True, przb_stacked=False, debug=False):
    import concourse.mybir as mybir
    import concourse.tile as tile
    from concourse import bacc

    F32, F32R = mybir.dt.float32, mybir.dt.float32r
    BF16 = mybir.dt.bfloat16
    AF = mybir.ActivationFunctionType

    nc = bacc.Bacc('TRN2', target_bir_lowering=False, debug=False, num_devices=8)
    dp = nc.declare_dram_parameter
    xt_d = dp("xt", [XP, E, XPW], BF16, isOutput=False)
    wg_d = dp("wg", [E, 3, 128], BF16, isOutput=False)
    wv_d = dp("wv", [E, 192], BF16, isOutput=False)
    wp_d = dp("wp", [2, 128, E], BF16, isOutput=False)
    bias_d = dp("bias", [128, 3], F32, isOutput=False)
    mk_d = dp("mk", [128, 19], F32, isOutput=False)
    ck_d = dp("ck", [128, SEQ], BF16, isOutput=False)
    sk_d = dp("sk", [128, SEQ], BF16, isOutput=False)
    out_d = dp("pout", [SQ, E], BF16, isOutput=True)
    if debug:
        dbg_d = dp("dbg", [128, 9, SEQ], BF16, isOutput=True)
        dbgv_d = dp("dbgv", [128, 19, HG, 65], BF16, isOutput=True)

    with tile.TileContext(nc) as tc:
        with (
            tc.tile_pool(name="long", bufs=1) as lp,
            tc.tile_pool(name="zp", bufs=2) as zp,
        ):
            # ---- long-lived SBUF ----
            # kt01 = [K-h0; K-h1] stacked, kq2 = [K-h2; Q-h2/garbage].
            # Per-head Q buffers carry ZEROS in the other 64 rows, so every
            # score matmul is a full K=128 contraction (registers as busy on
            # the PE clock monitor; K=64 shapes silently throttle to 1.2GHz).
            kt01 = lp.tile([128, SEQ], BF16, tag="kt01")
            kq2 = lp.tile([128, SEQ], BF16, tag="kq2")
            qt0z = lp.tile([128, SQ], BF16, tag="qt0z")
            qt1z = lp.tile([128, SQ], BF16, tag="qt1z")
            qt2z = lp.tile([128, SQ], BF16, tag="qt2z")
            v_all = lp.tile([128, 19, HG, 65], BF16, tag="v_all")
            ck_sb = lp.tile([128, SEQ], BF16, tag="ck")
            sk_sb = lp.tile([128, SEQ], BF16, tag="sk")
            xt = lp.tile([128, EC, SEQ], BF16, tag="xt")
            wg_sb = lp.tile([128, EC, 3, 128], BF16, tag="wg")
            wv_sb = lp.tile([128, EC, 192], BF16, tag="wv")
            wp_sb = lp.tile([128, 2, E], BF16, tag="wp")
            bias_sb = lp.tile([128, 3], F32, tag="bias")
            mk_sb = lp.tile([128, 19], F32, tag="mk")
            ones64 = lp.tile([1, 64], F32R, tag="ones64")
            ones128b = lp.tile([1, 128], F32R, tag="ones128b")
            ctxn01 = lp.tile([128, SQ], BF16, tag="ctxn01")
            ctxn2 = lp.tile([128, SQ], BF16, tag="ctxn2")
            wrm = lp.tile([128, 512], BF16, tag="wrm")
            wact = lp.tile([1, 8], F32, tag="wact")
            wact2 = lp.tile([1, 8], F32, tag="wact2")

            # ---- t=0: warm-up (PE clock + ACT tables) while DMAs run ----
            nc.vector.memset(wrm[:], 0.0)
            nc.vector.memset(wact[:], 0.0)
            nc.gpsimd.memset(ones64[:].bitcast(F32), 1.0)
            nc.gpsimd.memset(ones128b[0:1, 0:64].bitcast(F32), 0.0)
            nc.gpsimd.memset(ones128b[0:1, 64:128].bitcast(F32), 1.0)
            nc.gpsimd.memset(ctxn2[64:128, :], 0.0)
            nc.gpsimd.memset(qt0z[64:128, :], 0.0)
            nc.gpsimd.memset(qt1z[0:64, :], 0.0)
            nc.gpsimd.memset(qt2z[64:128, :], 0.0)
            with tc.tile_pool(name="wps", bufs=1, space="PSUM") as wps:
                wp_ps = wps.tile([128, 512], F32, tag="wps")
                for i in range(20):
                    nc.tensor.matmul(wp_ps[:, :], wrm[:, 0:128], wrm[:, 0:512],
                                     start=True, stop=True)
                # reader so the bank is only released after the last warm MM
                # (PE is FIFO, so this transitively orders all of them)
                nc.vector.tensor_copy(wact[:], wp_ps[0:1, 0:8])
            nc.scalar.activation(wact2[:], wact[:],
                                 AF.Exp, bias=0.0, scale=1.0)

            # ---- input DMAs, priority order ----
            nc.sync.dma_start(wv_sb[:], wv_d[:].rearrange("(c p) n -> p c n", c=EC))
            nc.sync.dma_start(bias_sb[:], bias_d[:])
            nc.sync.dma_start(mk_sb[:], mk_d[:])
            for pi in range(XP):
                nc.sync.dma_start(xt[:, :, pi * XPW:(pi + 1) * XPW],
                                  xt_d[pi].rearrange("(c p) n -> p c n", c=EC))
                if pi == 1:
                    nc.sync.dma_start(
                        wg_sb[:], wg_d[:].rearrange("(c p) g j -> p c g j", c=EC))
            nc.gpsimd.dma_start(ck_sb[:], ck_d[:])
            nc.gpsimd.dma_start(sk_sb[:], sk_d[:])
            nc.gpsimd.dma_start(wp_sb[:], wp_d[:].rearrange("t p n -> p t n"))

            # ---- phase A: V projection + K/Q projection with rope ----
            with tc.tile_pool(name="pk", bufs=2, space="PSUM") as pkp, \
                 tc.tile_pool(name="pv", bufs=2, space="PSUM") as pvp:

                def v_tile(i):
                    off, m = MC[i]
                    pv = pvp.tile([128, 192], F32, tag="pv", name="pv")
                    for c in range(EC):
                        nc.tensor.matmul(
                            pv[0:m, :], xt[:, c, off:off + m], wv_sb[:, c, :],
                            start=(c == 0), stop=(c == EC - 1))
                    nc.vector.tensor_mul(
                        v_all[0:m, i, :, 0:64],
                        pv[0:m, 0:192].rearrange("p (h d) -> p h d", h=HG),
                        mk_sb[0:m, i:i + 1].to_broadcast([m, HG, 64]))
                    nc.vector.tensor_copy(
                        v_all[0:m, i, :, 64:65],
                        mk_sb[0:m, i:i + 1].to_broadcast([m, HG, 1]))

                def g_tile(g, off, n):
                    # one M=128 projection tile for group g at cols off:off+n,
                    # bias-add + psum->sbuf copy on ScalarE, rope + partition-
                    # duplicated outputs on VectorE.
                    ps = pkp.tile([128, 512], F32, tag="pk", name="pk")
                    for c in range(EC):
                        nc.tensor.matmul(
                            ps[:, 0:n], wg_sb[:, c, g, :], xt[:, c, off:off + n],
                            start=(c == 0), stop=(c == EC - 1))
                    raw = zp.tile([128, 512], BF16, tag="raw", name="raw", bufs=2)
                    nc.scalar.activation(
                        raw[:, 0:n], ps[:, 0:n], AF.Identity,
                        bias=bias_sb[:, g:g + 1], scale=1.0)
                    t1 = zp.tile([128, 512], BF16, tag="rt1", name="rt1", bufs=2)
                    t2 = zp.tile([128, 512], BF16, tag="rt2", name="rt2", bufs=2)
                    nc.vector.tensor_mul(
                        t1[:, 0:n], raw[:, 0:n], ck_sb[:, off:off + n])
                    for b in range(4):
                        src = b * 32 + (32 if b % 2 == 0 else -32)
                        nc.vector.tensor_mul(
                            t2[b * 32:(b + 1) * 32, 0:n],
                            raw[src:src + 32, 0:n],
                            sk_sb[src:src + 32, off:off + n])
                    if g == 0:
                        nc.vector.tensor_add(
                            kt01[:, off:off + n], t1[:, 0:n], t2[:, 0:n])
                    elif g == 1:
                        nc.vector.tensor_add(
                            kq2[:, off:off + n], t1[:, 0:n], t2[:, 0:n])
                        if off < SQ:
                            n2 = min(n, SQ - off)
                            nc.vector.tensor_add(
                                qt2z[0:64, off:off + n2],
                                t1[64:128, 0:n2], t2[64:128, 0:n2])
                    else:
                        nc.vector.tensor_add(
                            qt0z[0:64, off:off + n], t1[0:64, 0:n], t2[0:64, 0:n])
                        nc.vector.tensor_add(
                            qt1z[64:128, off:off + n],
                            t1[64:128, 0:n], t2[64:128, 0:n])

                # G0 (K heads 0,1) + G2 (Q heads 0,1) first: they gate the
                # attention start.  V interleaved (xt column-progressive).
                # G1 (head 2) last: it overlaps the early attention phase.
                jobs = []
                for t in range(5):
                    jobs.append(("g", 0) + N_TILES[t])
                    if t < 3:
                        jobs.append(("g", 2) + L_TILES[t])
                vi = 0
                mixed = []
                for j, job in enumerate(jobs):
                    mixed.append(job)
                    while vi * len(jobs) < (j + 1) * 19:
                        mixed.append(("v", vi, 0, 0))
                        vi += 1
                while vi < 19:
                    mixed.append(("v", vi, 0, 0))
                    vi += 1
                for t in range(5):
                    mixed.append(("g", 1) + N_TILES[t])
                for kind, a, off, n in mixed:
                    if kind == "v":
                        v_tile(a)
                    else:
                        g_tile(a, off, n)
                # bridge the phase-A -> attention transition: dependency-free
                # full-array matmuls keep the HAM busy-signal alive while the
                # attention pipeline's first scores wait on the rope tail
                # (the PE clock otherwise re-throttles for ~3.4us here)
                tk_ps = pkp.tile([128, 512], F32, tag="pk", name="tkeep")
                for _ in range(8):
                    nc.tensor.matmul(tk_ps[:, 0:512], wrm[:, 0:128],
                                     wrm[:, 0:512], start=True, stop=True)
                nc.vector.tensor_copy(wact[:], tk_ps[0:1, 0:8])

            if debug:
                for di, t in enumerate([kt01, kq2]):
                    nc.sync.dma_start(dbg_d[:, di, :], t[:, :])
                for di, t in enumerate([qt0z, qt1z, qt2z]):
                    nc.sync.dma_start(dbg_d[:, 2 + di, 0:SQ], t[:, :])
                nc.sync.dma_start(dbgv_d[:], v_all[:])

            # ---- attention ----
            with tc.tile_pool(name="ep", bufs=3) as ep, \
                 tc.tile_pool(name="op", bufs=2) as op, \
                 tc.tile_pool(name="rzp", bufs=2) as rzp, \
                 tc.tile_pool(name="ps3", bufs=2, space="PSUM") as ps3, \
                 tc.tile_pool(name="pc3", bufs=1, space="PSUM") as pc3, \
                 tc.tile_pool(name="pk2", bufs=1, space="PSUM") as pkeep, \
                 tc.tile_pool(name="pp3", bufs=2, space="PSUM") as pp3:
                PROJ_OF_LT = {0: PT[0:4], 1: PT[4:7], 2: PT[7:10]}
                # L-tile-boundary HAM bridge: these must be emitted BEFORE
                # the dependency-stalled finish/proj matmuls — the PE issues
                # matmuls strictly in order, so keepers placed after them
                # (v8) just queue behind the stall and do nothing.
                kp_holder = []

                def keeper(n):
                    if not kp_holder:
                        kp_holder.append(pkeep.tile(
                            [128, 512], F32, tag="keep", name="keep"))
                    for _ in range(n):
                        nc.tensor.matmul(
                            kp_holder[0][:, 0:512], wrm[:, 0:128],
                            wrm[:, 0:512], start=True, stop=True)
                # K=128 scores: stationary carries BOTH heads' keys; the
                # per-head zero-padded Q buffer selects which half survives
                KT = {0: kt01, 1: kt01, 2: kq2}
                QT = {0: qt0z, 1: qt1z, 2: qt2z}

                def proj_slice(toff, tm):
                    outsb = op.tile([128, E], BF16, tag="outsb", name="outsb")
                    for half in range(2):
                        hs = half * 384
                        pp = pp3.tile([128, 512], F32, tag="pp", name="pp")
                        nc.tensor.matmul(
                            pp[0:tm, 0:384], ctxn01[:, toff:toff + tm],
                            wp_sb[:, 0, hs:hs + 384], start=True, stop=False)
                        nc.tensor.matmul(
                            pp[0:tm, 0:384], ctxn2[:, toff:toff + tm],
                            wp_sb[:, 1, hs:hs + 384], start=False, stop=True)
                        nc.vector.tensor_copy(outsb[0:tm, hs:hs + 384], pp[0:tm, 0:384])
                    nc.sync.dma_start(out_d[toff:toff + tm, :], outsb[0:tm, :])

                pending = []

                def rz_of(cu, ln2):
                    # 1/Z from the parked [65, ln] ctx+Z tile (Z = row 64);
                    # stage Z at partition 0 first (custom DVE ops want base 0)
                    zr = zp.tile([1, 512], F32, tag="zrow", name="zrow")
                    nc.vector.tensor_copy(zr[0:1, 0:ln2], cu[64:65, 0:ln2])
                    zscr = zp.tile([1, 512], F32, tag="zscr", name="zscr")
                    rzf = zp.tile([1, 512], F32, tag="rzf", name="rzf")
                    nc.vector.reciprocal_approx_accurate(
                        rzf[0:1, 0:ln2], zr[0:1, 0:ln2], zscr[0:1, 0:ln2])
                    rzr = zp.tile([1, 512], F32R, tag="rzr", name="rzr")
                    nc.vector.tensor_copy(rzr[0:1, 0:ln2], rzf[0:1, 0:ln2])
                    return rzr

                def finish01(z):
                    # joint normalize of heads 0+1 (stacked in ctxn01);
                    # normalize reads the broadcast psum directly.
                    cu0, cu1, loff2, ln2 = z
                    rz0, rz1 = rz_of(cu0, ln2), rz_of(cu1, ln2)
                    przb = pp3.tile([128, 512], F32, tag="pp", name="przb")
                    nc.tensor.matmul(
                        przb[:, 0:ln2], ones128b[:], rz1[0:1, 0:ln2],
                        start=True, stop=False)
                    nc.tensor.matmul(
                        przb[0:64, 0:ln2], ones64[:], rz0[0:1, 0:ln2],
                        start=False, stop=True, skip_group_check=True)
                    nc.vector.tensor_mul(
                        ctxn01[0:64, loff2:loff2 + ln2],
                        cu0[0:64, 0:ln2], przb[0:64, 0:ln2])
                    nc.vector.tensor_mul(
                        ctxn01[64:128, loff2:loff2 + ln2],
                        cu1[0:64, 0:ln2], przb[64:128, 0:ln2])

                def finish2(z):
                    cu2, loff2, ln2 = z
                    rzr = rz_of(cu2, ln2)
                    przb = pp3.tile([128, 512], F32, tag="pp", name="przb2")
                    nc.tensor.matmul(
                        przb[0:64, 0:ln2], ones64[:], rzr[0:1, 0:ln2],
                        start=True, stop=True)
                    nc.vector.tensor_mul(
                        ctxn2[0:64, loff2:loff2 + ln2], cu2[0:64, 0:ln2],
                        przb[0:64, 0:ln2])

                def drain_one():
                    if pending:
                        kind, z = pending.pop(0)
                        (finish01 if kind == 0 else finish2)(z)

                for lt_i, (loff, ln) in enumerate(L_TILES):
                    cus = []
                    for h in range(HG):
                        kth = KT[h]
                        qth = QT[h]
                        pctx = pc3.tile([65, 512], F32, tag="pctx")
                        exs = {}

                        def scores_exp(p):
                            chunks = GROUPS[p]
                            ps = ps3.tile([128, 2, 512], F32, tag="ps", name="ps")
                            for j, i in enumerate(chunks):
                                moff, m = MC[i]
                                nc.tensor.matmul(
                                    ps[0:m, j, 0:ln],
                                    kth[:, moff:moff + m],
                                    qth[:, loff:loff + ln],
                                    start=True, stop=True)
                            ex = ep.tile([128, 2, 512], BF16, tag="ex", name="ex")
                            m0 = MC[chunks[0]][1]
                            ng = len(chunks)
                            nc.scalar.activation(
                                ex[0:m0, 0:ng, 0:ln], ps[0:m0, 0:ng, 0:ln],
                                AF.Exp, bias=0.0, scale=SCALE)
                            exs[p] = ex

                        def ctx_mm(p):
                            ex = exs.pop(p)
                            for j, i in enumerate(GROUPS[p]):
                                moff, m = MC[i]
                                nc.tensor.matmul(
                                    pctx[:, 0:ln], v_all[0:m, i, h, :],
                                    ex[0:m, j, 0:ln],
                                    start=(i == 0), stop=(i == len(MC) - 1))

                        for p in range(len(GROUPS) + 2):
                            if p < len(GROUPS):
                                scores_exp(p)
                            if p == 2:
                                drain_one()
                            if p >= 2:
                                ctx_mm(p - 2)

                        # park ctx+Z ([65, ln]) in one copy; defer the rest
                        cu = rzp.tile([65, 512], F32, tag="cu65", name="cu65",
                                      bufs=3)
                        nc.vector.tensor_copy(cu[0:64, 0:ln], pctx[0:64, 0:ln])
                        nc.vector.tensor_copy(cu[64:65, 0:ln], pctx[64:65, 0:ln])
                        cus.append(cu)
                        if h == 1:
                            pending.append((0, (cus[0], cus[1], loff, ln)))
                        elif h == 2:
                            pending.append((1, (cu, loff, ln)))
                    if lt_i < len(L_TILES) - 1:
                        keeper(10)
                    while pending:
                        drain_one()
                    for (toff, tm) in PROJ_OF_LT[lt_i]:
                        proj_slice(toff, tm)
                if debug:
                    nc.sync.dma_start(dbg_d[:, 7, 0:SQ], ctxn01[:, :])
                    nc.sync.dma_start(dbg_d[:, 8, 0:SQ], ctxn2[:, :])

    nc.finalize()
    return nc


def _rope_tables():
    dim = D // 2
    freqs = 1.0 / 10000 ** (np.arange(0, dim, 2, dtype=np.float64) / dim)
    t = np.arange(GRID, dtype=np.float64)
    f = np.repeat(np.outer(t, freqs), 2, axis=-1)                  # [48, 32]
    fr = np.broadcast_to(f[:, None, :], (GRID, GRID, dim))
    fc = np.broadcast_to(f[None, :, :], (GRID, GRID, dim))
    full = np.concatenate([fr, fc], axis=-1).reshape(GRID * GRID, D)
    cos = np.ones((SEQ, D), np.float64)
    sin = np.zeros((SEQ, D), np.float64)
    cos[TASK:] = np.cos(full)
    sin[TASK:] = np.sin(full)
    return cos.astype(np.float32), sin.astype(np.float32)


def _signed_stack(tT):
    # [64, S] -> [128, S]: signed sine table stored at the ROTATED (source)
    # rows, so the rope half-multiplies read both operands at equal partition
    # bases: sinB[32:64] = -sin[0:32], sinB[0:32] = +sin[32:64], stacked x2.
    s = np.vstack([tT[32:64], -tT[0:32]])
    return np.ascontiguousarray(np.vstack([s, s]))


def _core_inputs(x, mask, Wqkv, Wproj, bqkv, cos, sin, g, s):
    import ml_dtypes
    bf = ml_dtypes.bfloat16
    xT = x.T  # [768, 2320]
    if s == 0:
        perm = None
        xt = np.ascontiguousarray(xT)
    else:
        perm = np.concatenate([np.arange(SQ, SEQ), np.arange(0, SQ)])
        xt = np.ascontiguousarray(np.concatenate([xT[:, SQ:], xT[:, :SQ]], axis=1))
    r0 = 192 * g
    wk = Wqkv[768 + r0:768 + r0 + 192, :].T          # [768, 192]
    wq = Wqkv[r0:r0 + 192, :].T
    wv = np.ascontiguousarray(Wqkv[1536 + r0:1536 + r0 + 192, :].T)
    # wg: [768, 3, 128]: g0 = K heads01; g1 = [K h2 | Q h2]; g2 = Q heads01
    wg = np.empty((E, 3, 128), np.float32)
    wg[:, 0, :] = wk[:, 0:128]
    wg[:, 1, 0:64] = wk[:, 128:192]
    wg[:, 1, 64:128] = wq[:, 128:192]
    wg[:, 2, :] = wq[:, 0:128]
    # bias columns (per-partition): same grouping
    bk = bqkv[768 + r0:768 + r0 + 192]
    bq = bqkv[r0:r0 + 192]
    bias = np.zeros((128, 3), np.float32)
    bias[:, 0] = bk[0:128]
    bias[0:64, 1] = bk[128:192]
    bias[64:128, 1] = bq[128:192]
    bias[:, 2] = bq[0:128]
    # wp: [2, 128, 768]: piece 0 = heads 0,1 d-rows; piece 1 = head 2 + zeros
    wpT = Wproj[:, r0:r0 + 192].T                    # [192, 768]
    wp = np.zeros((2, 128, E), np.float32)
    wp[0] = wpT[0:128]
    wp[1, 0:64] = wpT[128:192]
    cosT, sinT = cos.T, sin.T  # [64, S]
    ckf = np.vstack([cosT, cosT])
    skf = _signed_stack(sinT)
    if perm is not None:
        ckf = ckf[:, perm]
        skf = skf[:, perm]
    mk = mask.astype(np.float32)
    if perm is not None:
        mk = mk[perm]
    mk = np.concatenate([mk, np.zeros(19 * 128 - SEQ, np.float32)])
    mk = np.ascontiguousarray(mk.reshape(19, 128).T)
    return {
        "xt": np.ascontiguousarray(
            np.stack([xt[:, i * XPW:(i + 1) * XPW] for i in range(XP)])
        ).astype(bf),
        "wg": np.ascontiguousarray(wg).astype(bf),
        "wv": wv.astype(bf),
        "wp": np.ascontiguousarray(wp).astype(bf),
        "bias": bias,
        "mk": np.ascontiguousarray(mk),
        "ck": np.ascontiguousarray(ckf).astype(bf),
        "sk": np.ascontiguousarray(skf).astype(bf),
    }


def _run(x, mask, Wqkv, bqkv, Wproj, bproj, trace=False):
    global _prog
    from concourse.bass_utils import run_bass_kernel_spmd
    if _prog is None:
        _prog = _build()
    x = np.asarray(x, np.float32)
    mask = np.asarray(mask)
    Wqkv = np.asarray(Wqkv, np.float32)
    bqkv = np.asarray(bqkv, np.float32)
    Wproj = np.asarray(Wproj, np.float32)
    bproj = np.asarray(bproj, np.float32)
    cos, sin = _rope_tables()
    in_maps = [
        _core_inputs(x, mask, Wqkv, Wproj, bqkv, cos, sin, core // 2, core % 2)
        for core in range(8)
    ]
    res = run_bass_kernel_spmd(_prog, in_maps, list(range(8)), trace=trace)
    acc = np.zeros((SEQ, E), np.float64)
    for core in range(8):
        s = core % 2
        acc[SQ * s:SQ * (s + 1)] += res.results[core]["pout"].astype(np.float64)
    bias_row = bproj.astype(np.float64) + Wproj.astype(np.float64) @ \
        bqkv[1536:2304].astype(np.float64)
    acc += bias_row
    return acc.astype(np.float32), res


def kernel(x, mask, Wqkv, bqkv, Wproj, bproj):
    out, _ = _run(x, mask, Wqkv, bqkv, Wproj, bproj, trace=False)
    return out


# revision 53
# speedup vs baseline: 1.0375x; 1.0375x over previous
"""Multi-head attention (2D-RoPE, masked softmax) on 8 Trainium2 NeuronCores.

Sharding: 4 head-groups (3 heads each) x 2 query-halves (1160 rows each).
Each core computes full attention for its 3 heads over its 1160 query rows
against all 2320 keys, plus its share of the output projection; the host
sums the 8 partial projections and adds the (folded) biases.

v2 changes vs the first working kernel (164.6us):
  - DMA head: inputs arrive in 15 consolidated DMAs (priority-ordered,
    rearranged-on-read) instead of ~52; cq/sq tables dropped entirely
    (the permuted ck/sk's first 1160 columns ARE the query tables);
    Wproj and the output partials are bf16.  PE warm-up matmuls +
    an exp-table-load dummy run at t=0 so the HAM clock is at 2.4GHz
    and the ACT tables are resident when real work arrives.
  - Scores: K=64 row-tiled matmul PAIRS (tile_position (0,0)/(64,0) via
    operand base partitions) run two key-chunks concurrently on the two
    64-row halves of the PE array -- needs K^T and Q duplicated on
    partitions 64:128 (done with two extra [64,n] adds in the rope).
  - QKV: head2's K-rows and Q-rows are stacked into one M=128 stationary
    (G1) so its projection streams the sequence once, not twice.
  - exp in chunk-TRIPLES ([128,3,512] psum groups) -> 21 instead of 30
    ACTIVATE instructions per head.
  - proj: heads 0,1 ctx stacked into one [128, SQ] buffer -> K=128
    contraction (2 matmuls per half-slice instead of 3); their 1/Z
    broadcasts share one psum via col-tiled ones-matmuls.
"""
import sys
if '/opt/trn_rl_repo' not in sys.path:
    sys.path.insert(0, '/opt/trn_rl_repo')
import numpy as np

SEQ, E, NH, D = 2320, 768, 12, 64
GRID, TASK = 48, 16
SQ = SEQ // 2           # query rows per core
HG = 3                  # heads per core
SCALE = D ** -0.5
EC = 6                  # embed chunks of 128
L_TILES = [(0, 512), (512, 512), (1024, 136)]
N_TILES = [(0, 512), (512, 512), (1024, 512), (1536, 512), (2048, 272)]
MC = [(i * 128, min(128, SEQ - i * 128)) for i in range(19)]
PT = [(i * 128, min(128, SQ - i * 128)) for i in range(10)]
GROUPS = [tuple(range(g, min(g + 2, 19))) for g in range(0, 19, 2)]
XP = 8                  # xt column pieces
XPW = SEQ // XP         # 290

_prog = None


def _build(scores_tiled=True, przb_stacked=False, debug=False):
    import concourse.mybir as mybir
    import concourse.tile as tile
    from concourse import bacc

    F32, F32R = mybir.dt.float32, mybir.dt.float32r
    BF16 = mybir.dt.bfloat16
    AF = mybir.ActivationFunctionType

    nc = bacc.Bacc('TRN2', target_bir_lowering=False, debug=False, num_devices=8)
    dp = nc.declare_dram_parameter
    xt_d = dp("xt", [XP, E, XPW], BF16, isOutput=False)
    wg_d = dp("wg", [E, 3, 128], BF16, isOutput=False)
    wv_d = dp("wv", [E, 192], BF16, isOutput=False)
    wp_d = dp("wp", [2, 128, E], BF16, isOutput=False)
    bias_d = dp("bias", [128, 3], F32, isOutput=False)
    mk_d = dp("mk", [128, 19], F32, isOutput=False)
    ck_d = dp("ck", [128, SEQ], BF16, isOutput=False)
    sk_d = dp("sk", [128, SEQ], BF16, isOutput=False)
    out_d = dp("pout", [SQ, E], BF16, isOutput=True)
    if debug:
        dbg_d = dp("dbg", [128, 9, SEQ], BF16, isOutput=True)
        dbgv_d = dp("dbgv", [128, 19, HG, 65], BF16, isOutput=True)

    with tile.TileContext(nc) as tc:
        with (
            tc.tile_pool(name="long", bufs=1) as lp,
            tc.tile_pool(name="zp", bufs=2) as zp,
        ):
            # ---- long-lived SBUF ----
            # kt01 = [K-h0; K-h1] stacked, kq2 = [K-h2; Q-h2/garbage].
            # Per-head Q buffers carry ZEROS in the other 64 rows, so every
            # score matmul is a full K=128 contraction (registers as busy on
            # the PE clock monitor; K=64 shapes silently throttle to 1.2GHz).
            kt01 = lp.tile([128, SEQ], BF16, tag="kt01")
            kq2 = lp.tile([128, SEQ], BF16, tag="kq2")
            qt0z = lp.tile([128, SQ], BF16, tag="qt0z")
            qt1z = lp.tile([128, SQ], BF16, tag="qt1z")
            qt2z = lp.tile([128, SQ], BF16, tag="qt2z")
            v_all = lp.tile([128, 19, HG, 65], BF16, tag="v_all")
            ck_sb = lp.tile([128, SEQ], BF16, tag="ck")
            sk_sb = lp.tile([128, SEQ], BF16, tag="sk")
            xt = lp.tile([128, EC, SEQ], BF16, tag="xt")
            wg_sb = lp.tile([128, EC, 3, 128], BF16, tag="wg")
            wv_sb = lp.tile([128, EC, 192], BF16, tag="wv")
            wp_sb = lp.tile([128, 2, E], BF16, tag="wp")
            bias_sb = lp.tile([128, 3], F32, tag="bias")
            mk_sb = lp.tile([128, 19], F32, tag="mk")
            ones64 = lp.tile([1, 64], F32R, tag="ones64")
            ones128b = lp.tile([1, 128], F32R, tag="ones128b")
            ctxn01 = lp.tile([128, SQ], BF16, tag="ctxn01")
            ctxn2 = lp.tile([128, SQ], BF16, tag="ctxn2")
            wrm = lp.tile([128, 512], BF16, tag="wrm")
            wact = lp.tile([1, 8], F32, tag="wact")
            wact2 = lp.tile([1, 8], F32, tag="wact2")

            # ---- t=0: warm-up (PE clock + ACT tables) while DMAs run ----
            nc.vector.memset(wrm[:], 0.0)
            nc.vector.memset(wact[:], 0.0)
            nc.gpsimd.memset(ones64[:].bitcast(F32), 1.0)
            nc.gpsimd.memset(ones128b[0:1, 0:64].bitcast(F32), 0.0)
            nc.gpsimd.memset(ones128b[0:1, 64:128].bitcast(F32), 1.0)
            nc.gpsimd.memset(ctxn2[64:128, :], 0.0)
            nc.gpsimd.memset(qt0z[64:128, :], 0.0)
            nc.gpsimd.memset(qt1z[0:64, :], 0.0)
            nc.gpsimd.memset(qt2z[64:128, :], 0.0)
            with tc.tile_pool(name="wps", bufs=1, space="PSUM") as wps:
                wp_ps = wps.tile([128, 512], F32, tag="wps")
                for i in range(20):
                    nc.tensor.matmul(wp_ps[:, :], wrm[:, 0:128], wrm[:, 0:512],
                                     start=True, stop=True)
                # reader so the bank is only released after the last warm MM
                # (PE is FIFO, so this transitively orders all of them)
                nc.vector.tensor_copy(wact[:], wp_ps[0:1, 0:8])
            nc.scalar.activation(wact2[:], wact[:],
                                 AF.Exp, bias=0.0, scale=1.0)

            # ---- input DMAs, priority order ----
            nc.sync.dma_start(wv_sb[:], wv_d[:].rearrange("(c p) n -> p c n", c=EC))
            nc.sync.dma_start(bias_sb[:], bias_d[:])
            nc.sync.dma_start(mk_sb[:], mk_d[:])
            for pi in range(XP):
                nc.sync.dma_start(xt[:, :, pi * XPW:(pi + 1) * XPW],
                                  xt_d[pi].rearrange("(c p) n -> p c n", c=EC))
                if pi == 1:
                    nc.sync.dma_start(
                        wg_sb[:], wg_d[:].rearrange("(c p) g j -> p c g j", c=EC))
            nc.gpsimd.dma_start(ck_sb[:], ck_d[:])
            nc.gpsimd.dma_start(sk_sb[:], sk_d[:])
            nc.gpsimd.dma_start(wp_sb[:], wp_d[:].rearrange("t p n -> p t n"))

            # ---- phase A: V projection + K/Q projection with rope ----
            with tc.tile_pool(name="pk", bufs=2, space="PSUM") as pkp, \
                 tc.tile_pool(name="pv", bufs=2, space="PSUM") as pvp:

                def v_tile(i):
                    off, m = MC[i]
                    pv = pvp.tile([128, 192], F32, tag="pv", name="pv")
                    for c in range(EC):
                        nc.tensor.matmul(
                            pv[0:m, :], xt[:, c, off:off + m], wv_sb[:, c, :],
                            start=(c == 0), stop=(c == EC - 1))
                    nc.vector.tensor_mul(
                        v_all[0:m, i, :, 0:64],
                        pv[0:m, 0:192].rearrange("p (h d) -> p h d", h=HG),
                        mk_sb[0:m, i:i + 1].to_broadcast([m, HG, 64]))
                    nc.vector.tensor_copy(
                        v_all[0:m, i, :, 64:65],
                        mk_sb[0:m, i:i + 1].to_broadcast([m, HG, 1]))

                def g_tile(g, off, n):
                    # one M=128 projection tile for group g at cols off:off+n,
                    # bias-add + psum->sbuf copy on ScalarE, rope + partition-
                    # duplicated outputs on VectorE.
                    ps = pkp.tile([128, 512], F32, tag="pk", name="pk")
                    for c in range(EC):
                        nc.tensor.matmul(
                            ps[:, 0:n], wg_sb[:, c, g, :], xt[:, c, off:off + n],
                            start=(c == 0), stop=(c == EC - 1))
                    raw = zp.tile([128, 512], BF16, tag="raw", name="raw", bufs=2)
                    nc.scalar.activation(
                        raw[:, 0:n], ps[:, 0:n], AF.Identity,
                        bias=bias_sb[:, g:g + 1], scale=1.0)
                    t1 = zp.tile([128, 512], BF16, tag="rt1", name="rt1", bufs=2)
                    t2 = zp.tile([128, 512], BF16, tag="rt2", name="rt2", bufs=2)
                    nc.vector.tensor_mul(
                        t1[:, 0:n], raw[:, 0:n], ck_sb[:, off:off + n])
                    for b in range(4):
                        src = b * 32 + (32 if b % 2 == 0 else -32)
                        nc.vector.tensor_mul(
                            t2[b * 32:(b + 1) * 32, 0:n],
                            raw[src:src + 32, 0:n],
                            sk_sb[src:src + 32, off:off + n])
                    if g == 0:
                        nc.vector.tensor_add(
                            kt01[:, off:off + n], t1[:, 0:n], t2[:, 0:n])
                    elif g == 1:
                        nc.vector.tensor_add(
                            kq2[:, off:off + n], t1[:, 0:n], t2[:, 0:n])
                        if off < SQ:
                            n2 = min(n, SQ - off)
                            nc.vector.tensor_add(
                                qt2z[0:64, off:off + n2],
                                t1[64:128, 0:n2], t2[64:128, 0:n2])
                    else:
                        nc.vector.tensor_add(
                            qt0z[0:64, off:off + n], t1[0:64, 0:n], t2[0:64, 0:n])
                        nc.vector.tensor_add(
                            qt1z[64:128, off:off + n],
                            t1[64:128, 0:n], t2[64:128, 0:n])

                # G0 (K heads 0,1) + G2 (Q heads 0,1) first: they gate the
                # attention start.  V interleaved (xt column-progressive).
                # G1 (head 2) last: it overlaps the early attention phase.
                jobs = []
                for t in range(5):
                    jobs.append(("g", 0) + N_TILES[t])
                    if t < 3:
                        jobs.append(("g", 2) + L_TILES[t])
                vi = 0
                mixed = []
                for j, job in enumerate(jobs):
                    mixed.append(job)
                    while vi * len(jobs) < (j + 1) * 19:
                        mixed.append(("v", vi, 0, 0))
                        vi += 1
                while vi < 19:
                    mixed.append(("v", vi, 0, 0))
                    vi += 1
                for t in range(5):
                    mixed.append(("g", 1) + N_TILES[t])
                for kind, a, off, n in mixed:
                    if kind == "v":
                        v_tile(a)
                    else:
                        g_tile(a, off, n)

            if debug:
                for di, t in enumerate([kt01, kq2]):
                    nc.sync.dma_start(dbg_d[:, di, :], t[:, :])
                for di, t in enumerate([qt0z, qt1z, qt2z]):
                    nc.sync.dma_start(dbg_d[:, 2 + di, 0:SQ], t[:, :])
                nc.sync.dma_start(dbgv_d[:], v_all[:])

            # ---- attention ----
            with tc.tile_pool(name="ep", bufs=3) as ep, \
                 tc.tile_pool(name="op", bufs=2) as op, \
                 tc.tile_pool(name="rzp", bufs=2) as rzp, \
                 tc.tile_pool(name="ps3", bufs=2, space="PSUM") as ps3, \
                 tc.tile_pool(name="pc3", bufs=1, space="PSUM") as pc3, \
                 tc.tile_pool(name="pp3", bufs=2, space="PSUM") as pp3:
                PROJ_OF_LT = {0: PT[0:4], 1: PT[4:8], 2: PT[8:10]}
                # K=128 scores: stationary carries BOTH heads' keys; the
                # per-head zero-padded Q buffer selects which half survives
                KT = {0: kt01, 1: kt01, 2: kq2}
                QT = {0: qt0z, 1: qt1z, 2: qt2z}

                def proj_slice(toff, tm):
                    outsb = op.tile([128, E], BF16, tag="outsb", name="outsb")
                    for half in range(2):
                        hs = half * 384
                        pp = pp3.tile([128, 512], F32, tag="pp", name="pp")
                        nc.tensor.matmul(
                            pp[0:tm, 0:384], ctxn01[:, toff:toff + tm],
                            wp_sb[:, 0, hs:hs + 384], start=True, stop=False)
                        nc.tensor.matmul(
                            pp[0:tm, 0:384], ctxn2[:, toff:toff + tm],
                            wp_sb[:, 1, hs:hs + 384], start=False, stop=True)
                        nc.vector.tensor_copy(outsb[0:tm, hs:hs + 384], pp[0:tm, 0:384])
                    nc.sync.dma_start(out_d[toff:toff + tm, :], outsb[0:tm, :])

                pending = []

                def rz_of(cu, ln2):
                    # 1/Z from the parked [65, ln] ctx+Z tile (Z = row 64);
                    # stage Z at partition 0 first (custom DVE ops want base 0)
                    zr = zp.tile([1, 512], F32, tag="zrow", name="zrow")
                    nc.vector.tensor_copy(zr[0:1, 0:ln2], cu[64:65, 0:ln2])
                    zscr = zp.tile([1, 512], F32, tag="zscr", name="zscr")
                    rzf = zp.tile([1, 512], F32, tag="rzf", name="rzf")
                    nc.vector.reciprocal_approx_accurate(
                        rzf[0:1, 0:ln2], zr[0:1, 0:ln2], zscr[0:1, 0:ln2])
                    rzr = zp.tile([1, 512], F32R, tag="rzr", name="rzr")
                    nc.vector.tensor_copy(rzr[0:1, 0:ln2], rzf[0:1, 0:ln2])
                    return rzr

                def finish01(z):
                    # joint normalize of heads 0+1 (stacked in ctxn01);
                    # normalize reads the broadcast psum directly.
                    cu0, cu1, loff2, ln2 = z
                    rz0, rz1 = rz_of(cu0, ln2), rz_of(cu1, ln2)
                    przb = pp3.tile([128, 512], F32, tag="pp", name="przb")
                    nc.tensor.matmul(
                        przb[:, 0:ln2], ones128b[:], rz1[0:1, 0:ln2],
                        start=True, stop=False)
                    nc.tensor.matmul(
                        przb[0:64, 0:ln2], ones64[:], rz0[0:1, 0:ln2],
                        start=False, stop=True, skip_group_check=True)
                    nc.vector.tensor_mul(
                        ctxn01[0:64, loff2:loff2 + ln2],
                        cu0[0:64, 0:ln2], przb[0:64, 0:ln2])
                    nc.vector.tensor_mul(
                        ctxn01[64:128, loff2:loff2 + ln2],
                        cu1[0:64, 0:ln2], przb[64:128, 0:ln2])

                def finish2(z):
                    cu2, loff2, ln2 = z
                    rzr = rz_of(cu2, ln2)
                    przb = pp3.tile([128, 512], F32, tag="pp", name="przb2")
                    nc.tensor.matmul(
                        przb[0:64, 0:ln2], ones64[:], rzr[0:1, 0:ln2],
                        start=True, stop=True)
                    nc.vector.tensor_mul(
                        ctxn2[0:64, loff2:loff2 + ln2], cu2[0:64, 0:ln2],
                        przb[0:64, 0:ln2])

                def drain_one():
                    if pending:
                        kind, z = pending.pop(0)
                        (finish01 if kind == 0 else finish2)(z)

                for lt_i, (loff, ln) in enumerate(L_TILES):
                    cus = []
                    for h in range(HG):
                        kth = KT[h]
                        qth = QT[h]
                        pctx = pc3.tile([65, 512], F32, tag="pctx")
                        exs = {}

                        def scores_exp(p):
                            chunks = GROUPS[p]
                            ps = ps3.tile([128, 2, 512], F32, tag="ps", name="ps")
                            for j, i in enumerate(chunks):
                                moff, m = MC[i]
                                nc.tensor.matmul(
                                    ps[0:m, j, 0:ln],
                                    kth[:, moff:moff + m],
                                    qth[:, loff:loff + ln],
                                    start=True, stop=True)
                            ex = ep.tile([128, 2, 512], BF16, tag="ex", name="ex")
                            m0 = MC[chunks[0]][1]
                            ng = len(chunks)
                            nc.scalar.activation(
                                ex[0:m0, 0:ng, 0:ln], ps[0:m0, 0:ng, 0:ln],
                                AF.Exp, bias=0.0, scale=SCALE)
                            exs[p] = ex

                        def ctx_mm(p):
                            ex = exs.pop(p)
                            for j, i in enumerate(GROUPS[p]):
                                moff, m = MC[i]
                                nc.tensor.matmul(
                                    pctx[:, 0:ln], v_all[0:m, i, h, :],
                                    ex[0:m, j, 0:ln],
                                    start=(i == 0), stop=(i == len(MC) - 1))

                        for p in range(len(GROUPS) + 2):
                            if p < len(GROUPS):
                                scores_exp(p)
                            if p == 2:
                                drain_one()
                            if p >= 2:
                                ctx_mm(p - 2)

                        # park ctx+Z ([65, ln]) in one copy; defer the rest
                        cu = rzp.tile([65, 512], F32, tag="cu65", name="cu65",
                                      bufs=3)
                        nc.vector.tensor_copy(cu[0:64, 0:ln], pctx[0:64, 0:ln])
                        nc.vector.tensor_copy(cu[64:65, 0:ln], pctx[64:65, 0:ln])
                        cus.append(cu)
                        if h == 1:
                            pending.append((0, (cus[0], cus[1], loff, ln)))
                        elif h == 2:
                            pending.append((1, (cu, loff, ln)))
                    while pending:
                        drain_one()
                    for (toff, tm) in PROJ_OF_LT[lt_i]:
                        proj_slice(toff, tm)
                if debug:
                    nc.sync.dma_start(dbg_d[:, 7, 0:SQ], ctxn01[:, :])
                    nc.sync.dma_start(dbg_d[:, 8, 0:SQ], ctxn2[:, :])

    nc.finalize()
    return nc


def _rope_tables():
    dim = D // 2
    freqs = 1.0 / 10000 ** (np.arange(0, dim, 2, dtype=np.float64) / dim)
    t = np.arange(GRID, dtype=np.float64)
    f = np.repeat(np.outer(t, freqs), 2, axis=-1)                  # [48, 32]
    fr = np.broadcast_to(f[:, None, :], (GRID, GRID, dim))
    fc = np.broadcast_to(f[None, :, :], (GRID, GRID, dim))
    full = np.concatenate([fr, fc], axis=-1).reshape(GRID * GRID, D)
    cos = np.ones((SEQ, D), np.float64)
    sin = np.zeros((SEQ, D), np.float64)
    cos[TASK:] = np.cos(full)
    sin[TASK:] = np.sin(full)
    return cos.astype(np.float32), sin.astype(np.float32)


def _signed_stack(tT):
    # [64, S] -> [128, S]: signed sine table stored at the ROTATED (source)
    # rows, so the rope half-multiplies read both operands at equal partition
    # bases: sinB[32:64] = -sin[0:32], sinB[0:32] = +sin[32:64], stacked x2.
    s = np.vstack([tT[32:64], -tT[0:32]])
    return np.ascontiguousarray(np.vstack([s, s]))


def _core_inputs(x, mask, Wqkv, Wproj, bqkv, cos, sin, g, s):
    import ml_dtypes
    bf = ml_dtypes.bfloat16
    xT = x.T  # [768, 2320]
    if s == 0:
        perm = None
        xt = np.ascontiguousarray(xT)
    else:
        perm = np.concatenate([np.arange(SQ, SEQ), np.arange(0, SQ)])
        xt = np.ascontiguousarray(np.concatenate([xT[:, SQ:], xT[:, :SQ]], axis=1))
    r0 = 192 * g
    wk = Wqkv[768 + r0:768 + r0 + 192, :].T          # [768, 192]
    wq = Wqkv[r0:r0 + 192, :].T
    wv = np.ascontiguousarray(Wqkv[1536 + r0:1536 + r0 + 192, :].T)
    # wg: [768, 3, 128]: g0 = K heads01; g1 = [K h2 | Q h2]; g2 = Q heads01
    wg = np.empty((E, 3, 128), np.float32)
    wg[:, 0, :] = wk[:, 0:128]
    wg[:, 1, 0:64] = wk[:, 128:192]
    wg[:, 1, 64:128] = wq[:, 128:192]
    wg[:, 2, :] = wq[:, 0:128]
    # bias columns (per-partition): same grouping
    bk = bqkv[768 + r0:768 + r0 + 192]
    bq = bqkv[r0:r0 + 192]
    bias = np.zeros((128, 3), np.float32)
    bias[:, 0] = bk[0:128]
    bias[0:64, 1] = bk[128:192]
    bias[64:128, 1] = bq[128:192]
    bias[:, 2] = bq[0:128]
    # wp: [2, 128, 768]: piece 0 = heads 0,1 d-rows; piece 1 = head 2 + zeros
    wpT = Wproj[:, r0:r0 + 192].T                    # [192, 768]
    wp = np.zeros((2, 128, E), np.float32)
    wp[0] = wpT[0:128]
    wp[1, 0:64] = wpT[128:192]
    cosT, sinT = cos.T, sin.T  # [64, S]
    ckf = np.vstack([cosT, cosT])
    skf = _signed_stack(sinT)
    if perm is not None:
        ckf = ckf[:, perm]
        skf = skf[:, perm]
    mk = mask.astype(np.float32)
    if perm is not None:
        mk = mk[perm]
    mk = np.concatenate([mk, np.zeros(19 * 128 - SEQ, np.float32)])
    mk = np.ascontiguousarray(mk.reshape(19, 128).T)
    return {
        "xt": np.ascontiguousarray(
            np.stack([xt[:, i * XPW:(i + 1) * XPW] for i in range(XP)])
        ).astype(bf),
        "wg": np.ascontiguousarray(wg).astype(bf),
        "wv": wv.astype(bf),
        "wp": np.ascontiguousarray(wp).astype(bf),
        "bias": bias,
        "mk": np.ascontiguousarray(mk),
        "ck": np.ascontiguousarray(ckf).astype(bf),
        "sk": np.ascontiguousarray(skf).astype(bf),
    }


def _run(x, mask, Wqkv, bqkv, Wproj, bproj, trace=False):
    global _prog
    from concourse.bass_utils import run_bass_kernel_spmd
    if _prog is None:
        _prog = _build()
    x = np.asarray(x, np.float32)
    mask = np.asarray(mask)
    Wqkv = np.asarray(Wqkv, np.float32)
    bqkv = np.asarray(bqkv, np.float32)
    Wproj = np.asarray(Wproj, np.float32)
    bproj = np.asarray(bproj, np.float32)
    cos, sin = _rope_tables()
    in_maps = [
        _core_inputs(x, mask, Wqkv, Wproj, bqkv, cos, sin, core // 2, core % 2)
        for core in range(8)
    ]
    res = run_bass_kernel_spmd(_prog, in_maps, list(range(8)), trace=trace)
    acc = np.zeros((SEQ, E), np.float64)
    for core in range(8):
        s = core % 2
        acc[SQ * s:SQ * (s + 1)] += res.results[core]["pout"].astype(np.float64)
    bias_row = bproj.astype(np.float64) + Wproj.astype(np.float64) @ \
        bqkv[1536:2304].astype(np.float64)
    acc += bias_row
    return acc.astype(np.float32), res


def kernel(x, mask, Wqkv, bqkv, Wproj, bproj):
    out, _ = _run(x, mask, Wqkv, bqkv, Wproj, bproj, trace=False)
    return out


# revision 54
# speedup vs baseline: 1.0648x; 1.0263x over previous
"""Multi-head attention (2D-RoPE, masked softmax) on 8 Trainium2 NeuronCores.

Sharding: 4 head-groups (3 heads each) x 2 query-halves (1160 rows each).
Each core computes full attention for its 3 heads over its 1160 query rows
against all 2320 keys, plus its share of the output projection; the host
sums the 8 partial projections and adds the (folded) biases.

v2 changes vs the first working kernel (164.6us):
  - DMA head: inputs arrive in 15 consolidated DMAs (priority-ordered,
    rearranged-on-read) instead of ~52; cq/sq tables dropped entirely
    (the permuted ck/sk's first 1160 columns ARE the query tables);
    Wproj and the output partials are bf16.  PE warm-up matmuls +
    an exp-table-load dummy run at t=0 so the HAM clock is at 2.4GHz
    and the ACT tables are resident when real work arrives.
  - Scores: K=64 row-tiled matmul PAIRS (tile_position (0,0)/(64,0) via
    operand base partitions) run two key-chunks concurrently on the two
    64-row halves of the PE array -- needs K^T and Q duplicated on
    partitions 64:128 (done with two extra [64,n] adds in the rope).
  - QKV: head2's K-rows and Q-rows are stacked into one M=128 stationary
    (G1) so its projection streams the sequence once, not twice.
  - exp in chunk-TRIPLES ([128,3,512] psum groups) -> 21 instead of 30
    ACTIVATE instructions per head.
  - proj: heads 0,1 ctx stacked into one [128, SQ] buffer -> K=128
    contraction (2 matmuls per half-slice instead of 3); their 1/Z
    broadcasts share one psum via col-tiled ones-matmuls.
"""
import sys
if '/opt/trn_rl_repo' not in sys.path:
    sys.path.insert(0, '/opt/trn_rl_repo')
import numpy as np

SEQ, E, NH, D = 2320, 768, 12, 64
GRID, TASK = 48, 16
SQ = SEQ // 2           # query rows per core
HG = 3                  # heads per core
SCALE = D ** -0.5
EC = 6                  # embed chunks of 128
L_TILES = [(0, 512), (512, 512), (1024, 136)]
N_TILES = [(0, 512), (512, 512), (1024, 512), (1536, 512), (2048, 272)]
MC = [(i * 128, min(128, SEQ - i * 128)) for i in range(19)]
PT = [(i * 128, min(128, SQ - i * 128)) for i in range(10)]
GROUPS = [tuple(range(g, min(g + 2, 19))) for g in range(0, 19, 2)]
XP = 8                  # xt column pieces
XPW = SEQ // XP         # 290

_prog = None


def _build(scores_tiled=True, przb_stacked=False, debug=False):
    import concourse.mybir as mybir
    import concourse.tile as tile
    from concourse import bacc

    F32, F32R = mybir.dt.float32, mybir.dt.float32r
    BF16 = mybir.dt.bfloat16
    AF = mybir.ActivationFunctionType

    nc = bacc.Bacc('TRN2', target_bir_lowering=False, debug=False, num_devices=8)
    dp = nc.declare_dram_parameter
    xt_d = dp("xt", [XP, E, XPW], BF16, isOutput=False)
    wg_d = dp("wg", [E, 3, 128], BF16, isOutput=False)
    wv_d = dp("wv", [E, 192], BF16, isOutput=False)
    wp_d = dp("wp", [2, 128, E], BF16, isOutput=False)
    bias_d = dp("bias", [128, 3], F32, isOutput=False)
    mk_d = dp("mk", [128, 19], F32, isOutput=False)
    ck_d = dp("ck", [128, SEQ], BF16, isOutput=False)
    sk_d = dp("sk", [128, SEQ], BF16, isOutput=False)
    out_d = dp("pout", [SQ, E], BF16, isOutput=True)
    if debug:
        dbg_d = dp("dbg", [128, 9, SEQ], BF16, isOutput=True)
        dbgv_d = dp("dbgv", [128, 19, HG, 65], BF16, isOutput=True)

    with tile.TileContext(nc) as tc:
        with (
            tc.tile_pool(name="long", bufs=1) as lp,
            tc.tile_pool(name="zp", bufs=2) as zp,
        ):
            # ---- long-lived SBUF ----
            # kt01 = [K-h0; K-h1] stacked, kq2 = [K-h2; Q-h2/garbage].
            # Per-head Q buffers carry ZEROS in the other 64 rows, so every
            # score matmul is a full K=128 contraction (registers as busy on
            # the PE clock monitor; K=64 shapes silently throttle to 1.2GHz).
            kt01 = lp.tile([128, SEQ], BF16, tag="kt01")
            kq2 = lp.tile([128, SEQ], BF16, tag="kq2")
            qt0z = lp.tile([128, SQ], BF16, tag="qt0z")
            qt1z = lp.tile([128, SQ], BF16, tag="qt1z")
            qt2z = lp.tile([128, SQ], BF16, tag="qt2z")
            v_all = lp.tile([128, 19, HG, 65], BF16, tag="v_all")
            ck_sb = lp.tile([128, SEQ], BF16, tag="ck")
            sk_sb = lp.tile([128, SEQ], BF16, tag="sk")
            xt = lp.tile([128, EC, SEQ], BF16, tag="xt")
            wg_sb = lp.tile([128, EC, 3, 128], BF16, tag="wg")
            wv_sb = lp.tile([128, EC, 192], BF16, tag="wv")
            wp_sb = lp.tile([128, 2, E], BF16, tag="wp")
            bias_sb = lp.tile([128, 3], F32, tag="bias")
            mk_sb = lp.tile([128, 19], F32, tag="mk")
            ones64 = lp.tile([1, 64], F32R, tag="ones64")
            ones128b = lp.tile([1, 128], F32R, tag="ones128b")
            ctxn01 = lp.tile([128, SQ], BF16, tag="ctxn01")
            ctxn2 = lp.tile([128, SQ], BF16, tag="ctxn2")
            wrm = lp.tile([128, 512], BF16, tag="wrm")
            wact = lp.tile([1, 8], F32, tag="wact")
            wact2 = lp.tile([1, 8], F32, tag="wact2")

            # ---- t=0: warm-up (PE clock + ACT tables) while DMAs run ----
            nc.vector.memset(wrm[:], 0.0)
            nc.vector.memset(wact[:], 0.0)
            nc.gpsimd.memset(ones64[:].bitcast(F32), 1.0)
            nc.gpsimd.memset(ones128b[0:1, 0:64].bitcast(F32), 0.0)
            nc.gpsimd.memset(ones128b[0:1, 64:128].bitcast(F32), 1.0)
            nc.gpsimd.memset(ctxn2[64:128, :], 0.0)
            nc.gpsimd.memset(qt0z[64:128, :], 0.0)
            nc.gpsimd.memset(qt1z[0:64, :], 0.0)
            nc.gpsimd.memset(qt2z[64:128, :], 0.0)
            with tc.tile_pool(name="wps", bufs=1, space="PSUM") as wps:
                wp_ps = wps.tile([128, 512], F32, tag="wps")
                for i in range(20):
                    nc.tensor.matmul(wp_ps[:, :], wrm[:, 0:128], wrm[:, 0:512],
                                     start=True, stop=True)
                # reader so the bank is only released after the last warm MM
                # (PE is FIFO, so this transitively orders all of them)
                nc.vector.tensor_copy(wact[:], wp_ps[0:1, 0:8])
            nc.scalar.activation(wact2[:], wact[:],
                                 AF.Exp, bias=0.0, scale=1.0)

            # ---- input DMAs, priority order ----
            nc.sync.dma_start(wv_sb[:], wv_d[:].rearrange("(c p) n -> p c n", c=EC))
            nc.sync.dma_start(bias_sb[:], bias_d[:])
            nc.sync.dma_start(mk_sb[:], mk_d[:])
            for pi in range(XP):
                nc.sync.dma_start(xt[:, :, pi * XPW:(pi + 1) * XPW],
                                  xt_d[pi].rearrange("(c p) n -> p c n", c=EC))
                if pi == 1:
                    nc.sync.dma_start(
                        wg_sb[:], wg_d[:].rearrange("(c p) g j -> p c g j", c=EC))
            nc.gpsimd.dma_start(ck_sb[:], ck_d[:])
            nc.gpsimd.dma_start(sk_sb[:], sk_d[:])
            nc.gpsimd.dma_start(wp_sb[:], wp_d[:].rearrange("t p n -> p t n"))

            # ---- phase A: V projection + K/Q projection with rope ----
            with tc.tile_pool(name="pk", bufs=2, space="PSUM") as pkp, \
                 tc.tile_pool(name="pv", bufs=2, space="PSUM") as pvp:

                def v_tile(i):
                    off, m = MC[i]
                    pv = pvp.tile([128, 192], F32, tag="pv", name="pv")
                    for c in range(EC):
                        nc.tensor.matmul(
                            pv[0:m, :], xt[:, c, off:off + m], wv_sb[:, c, :],
                            start=(c == 0), stop=(c == EC - 1))
                    # mask fold on ScalarE (per-partition scale AP): VectorE
                    # is the phase-A critical path (rope), ScalarE is idle
                    nc.scalar.activation(
                        v_all[0:m, i, :, 0:64],
                        pv[0:m, 0:192].rearrange("p (h d) -> p h d", h=HG),
                        AF.Identity, bias=0.0, scale=mk_sb[0:m, i:i + 1])
                    nc.vector.tensor_copy(
                        v_all[0:m, i, :, 64:65],
                        mk_sb[0:m, i:i + 1].to_broadcast([m, HG, 1]))

                def g_tile(g, off, n):
                    # one M=128 projection tile for group g at cols off:off+n,
                    # bias-add + psum->sbuf copy on ScalarE, rope + partition-
                    # duplicated outputs on VectorE.
                    ps = pkp.tile([128, 512], F32, tag="pk", name="pk")
                    for c in range(EC):
                        nc.tensor.matmul(
                            ps[:, 0:n], wg_sb[:, c, g, :], xt[:, c, off:off + n],
                            start=(c == 0), stop=(c == EC - 1))
                    raw = zp.tile([128, 512], BF16, tag="raw", name="raw", bufs=2)
                    nc.scalar.activation(
                        raw[:, 0:n], ps[:, 0:n], AF.Identity,
                        bias=bias_sb[:, g:g + 1], scale=1.0)
                    t1 = zp.tile([128, 512], BF16, tag="rt1", name="rt1", bufs=2)
                    t2 = zp.tile([128, 512], BF16, tag="rt2", name="rt2", bufs=2)
                    nc.vector.tensor_mul(
                        t1[:, 0:n], raw[:, 0:n], ck_sb[:, off:off + n])
                    for b in range(4):
                        src = b * 32 + (32 if b % 2 == 0 else -32)
                        nc.vector.tensor_mul(
                            t2[b * 32:(b + 1) * 32, 0:n],
                            raw[src:src + 32, 0:n],
                            sk_sb[src:src + 32, off:off + n])
                    if g == 0:
                        nc.vector.tensor_add(
                            kt01[:, off:off + n], t1[:, 0:n], t2[:, 0:n])
                    elif g == 1:
                        nc.vector.tensor_add(
                            kq2[:, off:off + n], t1[:, 0:n], t2[:, 0:n])
                        if off < SQ:
                            n2 = min(n, SQ - off)
                            nc.vector.tensor_add(
                                qt2z[0:64, off:off + n2],
                                t1[64:128, 0:n2], t2[64:128, 0:n2])
                    else:
                        nc.vector.tensor_add(
                            qt0z[0:64, off:off + n], t1[0:64, 0:n], t2[0:64, 0:n])
                        nc.vector.tensor_add(
                            qt1z[64:128, off:off + n],
                            t1[64:128, 0:n], t2[64:128, 0:n])

                # G0 (K heads 0,1) + G2 (Q heads 0,1) first: they gate the
                # attention start.  V interleaved (xt column-progressive).
                # G1 (head 2) last: it overlaps the early attention phase.
                jobs = []
                for t in range(5):
                    jobs.append(("g", 0) + N_TILES[t])
                    if t < 3:
                        jobs.append(("g", 2) + L_TILES[t])
                vi = 0
                mixed = []
                for j, job in enumerate(jobs):
                    mixed.append(job)
                    while vi * len(jobs) < (j + 1) * 19:
                        mixed.append(("v", vi, 0, 0))
                        vi += 1
                while vi < 19:
                    mixed.append(("v", vi, 0, 0))
                    vi += 1
                for t in range(5):
                    mixed.append(("g", 1) + N_TILES[t])
                for kind, a, off, n in mixed:
                    if kind == "v":
                        v_tile(a)
                    else:
                        g_tile(a, off, n)

            if debug:
                for di, t in enumerate([kt01, kq2]):
                    nc.sync.dma_start(dbg_d[:, di, :], t[:, :])
                for di, t in enumerate([qt0z, qt1z, qt2z]):
                    nc.sync.dma_start(dbg_d[:, 2 + di, 0:SQ], t[:, :])
                nc.sync.dma_start(dbgv_d[:], v_all[:])

            # ---- attention ----
            with tc.tile_pool(name="ep", bufs=3) as ep, \
                 tc.tile_pool(name="op", bufs=2) as op, \
                 tc.tile_pool(name="rzp", bufs=2) as rzp, \
                 tc.tile_pool(name="ps3", bufs=2, space="PSUM") as ps3, \
                 tc.tile_pool(name="pc3", bufs=1, space="PSUM") as pc3, \
                 tc.tile_pool(name="pp3", bufs=2, space="PSUM") as pp3:
                PROJ_OF_LT = {0: PT[0:4], 1: PT[4:8], 2: PT[8:10]}
                # K=128 scores: stationary carries BOTH heads' keys; the
                # per-head zero-padded Q buffer selects which half survives
                KT = {0: kt01, 1: kt01, 2: kq2}
                QT = {0: qt0z, 1: qt1z, 2: qt2z}

                def proj_slice(toff, tm):
                    outsb = op.tile([128, E], BF16, tag="outsb", name="outsb")
                    for half in range(2):
                        hs = half * 384
                        pp = pp3.tile([128, 512], F32, tag="pp", name="pp")
                        nc.tensor.matmul(
                            pp[0:tm, 0:384], ctxn01[:, toff:toff + tm],
                            wp_sb[:, 0, hs:hs + 384], start=True, stop=False)
                        nc.tensor.matmul(
                            pp[0:tm, 0:384], ctxn2[:, toff:toff + tm],
                            wp_sb[:, 1, hs:hs + 384], start=False, stop=True)
                        nc.vector.tensor_copy(outsb[0:tm, hs:hs + 384], pp[0:tm, 0:384])
                    nc.sync.dma_start(out_d[toff:toff + tm, :], outsb[0:tm, :])

                pending = []

                def rz_of(cu, ln2):
                    # 1/Z from the parked [65, ln] ctx+Z tile (Z = row 64);
                    # stage Z at partition 0 first (custom DVE ops want base 0)
                    zr = zp.tile([1, 512], F32, tag="zrow", name="zrow")
                    nc.vector.tensor_copy(zr[0:1, 0:ln2], cu[64:65, 0:ln2])
                    zscr = zp.tile([1, 512], F32, tag="zscr", name="zscr")
                    rzf = zp.tile([1, 512], F32, tag="rzf", name="rzf")
                    nc.vector.reciprocal_approx_accurate(
                        rzf[0:1, 0:ln2], zr[0:1, 0:ln2], zscr[0:1, 0:ln2])
                    rzr = zp.tile([1, 512], F32R, tag="rzr", name="rzr")
                    nc.vector.tensor_copy(rzr[0:1, 0:ln2], rzf[0:1, 0:ln2])
                    return rzr

                def finish01(z):
                    # joint normalize of heads 0+1 (stacked in ctxn01);
                    # normalize reads the broadcast psum directly.
                    cu0, cu1, loff2, ln2 = z
                    rz0, rz1 = rz_of(cu0, ln2), rz_of(cu1, ln2)
                    przb = pp3.tile([128, 512], F32, tag="pp", name="przb")
                    nc.tensor.matmul(
                        przb[:, 0:ln2], ones128b[:], rz1[0:1, 0:ln2],
                        start=True, stop=False)
                    nc.tensor.matmul(
                        przb[0:64, 0:ln2], ones64[:], rz0[0:1, 0:ln2],
                        start=False, stop=True, skip_group_check=True)
                    nc.vector.tensor_mul(
                        ctxn01[0:64, loff2:loff2 + ln2],
                        cu0[0:64, 0:ln2], przb[0:64, 0:ln2])
                    nc.vector.tensor_mul(
                        ctxn01[64:128, loff2:loff2 + ln2],
                        cu1[0:64, 0:ln2], przb[64:128, 0:ln2])

                def finish2(z):
                    cu2, loff2, ln2 = z
                    rzr = rz_of(cu2, ln2)
                    przb = pp3.tile([128, 512], F32, tag="pp", name="przb2")
                    nc.tensor.matmul(
                        przb[0:64, 0:ln2], ones64[:], rzr[0:1, 0:ln2],
                        start=True, stop=True)
                    nc.vector.tensor_mul(
                        ctxn2[0:64, loff2:loff2 + ln2], cu2[0:64, 0:ln2],
                        przb[0:64, 0:ln2])

                def drain_one():
                    if pending:
                        kind, z = pending.pop(0)
                        (finish01 if kind == 0 else finish2)(z)

                for lt_i, (loff, ln) in enumerate(L_TILES):
                    cus = []
                    for h in range(HG):
                        kth = KT[h]
                        qth = QT[h]
                        pctx = pc3.tile([65, 512], F32, tag="pctx")
                        exs = {}

                        def scores_exp(p):
                            chunks = GROUPS[p]
                            ps = ps3.tile([128, 2, 512], F32, tag="ps", name="ps")
                            for j, i in enumerate(chunks):
                                moff, m = MC[i]
                                nc.tensor.matmul(
                                    ps[0:m, j, 0:ln],
                                    kth[:, moff:moff + m],
                                    qth[:, loff:loff + ln],
                                    start=True, stop=True)
                            ex = ep.tile([128, 2, 512], BF16, tag="ex", name="ex")
                            m0 = MC[chunks[0]][1]
                            ng = len(chunks)
                            nc.scalar.activation(
                                ex[0:m0, 0:ng, 0:ln], ps[0:m0, 0:ng, 0:ln],
                                AF.Exp, bias=0.0, scale=SCALE)
                            exs[p] = ex

                        def ctx_mm(p):
                            ex = exs.pop(p)
                            for j, i in enumerate(GROUPS[p]):
                                moff, m = MC[i]
                                nc.tensor.matmul(
                                    pctx[:, 0:ln], v_all[0:m, i, h, :],
                                    ex[0:m, j, 0:ln],
                                    start=(i == 0), stop=(i == len(MC) - 1))

                        for p in range(len(GROUPS) + 2):
                            if p < len(GROUPS):
                                scores_exp(p)
                            if p == 2:
                                drain_one()
                            if p >= 2:
                                ctx_mm(p - 2)

                        # park ctx+Z ([65, ln]) in one copy; defer the rest
                        cu = rzp.tile([65, 512], F32, tag="cu65", name="cu65",
                                      bufs=3)
                        nc.vector.tensor_copy(cu[0:64, 0:ln], pctx[0:64, 0:ln])
                        nc.vector.tensor_copy(cu[64:65, 0:ln], pctx[64:65, 0:ln])
                        cus.append(cu)
                        if h == 1:
                            pending.append((0, (cus[0], cus[1], loff, ln)))
                        elif h == 2:
                            pending.append((1, (cu, loff, ln)))
                    while pending:
                        drain_one()
                    for (toff, tm) in PROJ_OF_LT[lt_i]:
                        proj_slice(toff, tm)
                if debug:
                    nc.sync.dma_start(dbg_d[:, 7, 0:SQ], ctxn01[:, :])
                    nc.sync.dma_start(dbg_d[:, 8, 0:SQ], ctxn2[:, :])

    nc.finalize()
    return nc


def _rope_tables():
    dim = D // 2
    freqs = 1.0 / 10000 ** (np.arange(0, dim, 2, dtype=np.float64) / dim)
    t = np.arange(GRID, dtype=np.float64)
    f = np.repeat(np.outer(t, freqs), 2, axis=-1)                  # [48, 32]
    fr = np.broadcast_to(f[:, None, :], (GRID, GRID, dim))
    fc = np.broadcast_to(f[None, :, :], (GRID, GRID, dim))
    full = np.concatenate([fr, fc], axis=-1).reshape(GRID * GRID, D)
    cos = np.ones((SEQ, D), np.float64)
    sin = np.zeros((SEQ, D), np.float64)
    cos[TASK:] = np.cos(full)
    sin[TASK:] = np.sin(full)
    return cos.astype(np.float32), sin.astype(np.float32)


def _signed_stack(tT):
    # [64, S] -> [128, S]: signed sine table stored at the ROTATED (source)
    # rows, so the rope half-multiplies read both operands at equal partition
    # bases: sinB[32:64] = -sin[0:32], sinB[0:32] = +sin[32:64], stacked x2.
    s = np.vstack([tT[32:64], -tT[0:32]])
    return np.ascontiguousarray(np.vstack([s, s]))


def _core_inputs(x, mask, Wqkv, Wproj, bqkv, cos, sin, g, s):
    import ml_dtypes
    bf = ml_dtypes.bfloat16
    xT = x.T  # [768, 2320]
    if s == 0:
        perm = None
        xt = np.ascontiguousarray(xT)
    else:
        perm = np.concatenate([np.arange(SQ, SEQ), np.arange(0, SQ)])
        xt = np.ascontiguousarray(np.concatenate([xT[:, SQ:], xT[:, :SQ]], axis=1))
    r0 = 192 * g
    wk = Wqkv[768 + r0:768 + r0 + 192, :].T          # [768, 192]
    wq = Wqkv[r0:r0 + 192, :].T
    wv = np.ascontiguousarray(Wqkv[1536 + r0:1536 + r0 + 192, :].T)
    # wg: [768, 3, 128]: g0 = K heads01; g1 = [K h2 | Q h2]; g2 = Q heads01
    wg = np.empty((E, 3, 128), np.float32)
    wg[:, 0, :] = wk[:, 0:128]
    wg[:, 1, 0:64] = wk[:, 128:192]
    wg[:, 1, 64:128] = wq[:, 128:192]
    wg[:, 2, :] = wq[:, 0:128]
    # bias columns (per-partition): same grouping
    bk = bqkv[768 + r0:768 + r0 + 192]
    bq = bqkv[r0:r0 + 192]
    bias = np.zeros((128, 3), np.float32)
    bias[:, 0] = bk[0:128]
    bias[0:64, 1] = bk[128:192]
    bias[64:128, 1] = bq[128:192]
    bias[:, 2] = bq[0:128]
    # wp: [2, 128, 768]: piece 0 = heads 0,1 d-rows; piece 1 = head 2 + zeros
    wpT = Wproj[:, r0:r0 + 192].T                    # [192, 768]
    wp = np.zeros((2, 128, E), np.float32)
    wp[0] = wpT[0:128]
    wp[1, 0:64] = wpT[128:192]
    cosT, sinT = cos.T, sin.T  # [64, S]
    ckf = np.vstack([cosT, cosT])
    skf = _signed_stack(sinT)
    if perm is not None:
        ckf = ckf[:, perm]
        skf = skf[:, perm]
    mk = mask.astype(np.float32)
    if perm is not None:
        mk = mk[perm]
    mk = np.concatenate([mk, np.zeros(19 * 128 - SEQ, np.float32)])
    mk = np.ascontiguousarray(mk.reshape(19, 128).T)
    return {
        "xt": np.ascontiguousarray(
            np.stack([xt[:, i * XPW:(i + 1) * XPW] for i in range(XP)])
        ).astype(bf),
        "wg": np.ascontiguousarray(wg).astype(bf),
        "wv": wv.astype(bf),
        "wp": np.ascontiguousarray(wp).astype(bf),
        "bias": bias,
        "mk": np.ascontiguousarray(mk),
        "ck": np.ascontiguousarray(ckf).astype(bf),
        "sk": np.ascontiguousarray(skf).astype(bf),
    }


def _run(x, mask, Wqkv, bqkv, Wproj, bproj, trace=False):
    global _prog
    from concourse.bass_utils import run_bass_kernel_spmd
    if _prog is None:
        _prog = _build()
    x = np.asarray(x, np.float32)
    mask = np.asarray(mask)
    Wqkv = np.asarray(Wqkv, np.float32)
    bqkv = np.asarray(bqkv, np.float32)
    Wproj = np.asarray(Wproj, np.float32)
    bproj = np.asarray(bproj, np.float32)
    cos, sin = _rope_tables()
    in_maps = [
        _core_inputs(x, mask, Wqkv, Wproj, bqkv, cos, sin, core // 2, core % 2)
        for core in range(8)
    ]
    res = run_bass_kernel_spmd(_prog, in_maps, list(range(8)), trace=trace)
    acc = np.zeros((SEQ, E), np.float64)
    for core in range(8):
        s = core % 2
        acc[SQ * s:SQ * (s + 1)] += res.results[core]["pout"].astype(np.float64)
    bias_row = bproj.astype(np.float64) + Wproj.astype(np.float64) @ \
        bqkv[1536:2304].astype(np.float64)
    acc += bias_row
    return acc.astype(np.float32), res


def kernel(x, mask, Wqkv, bqkv, Wproj, bproj):
    out, _ = _run(x, mask, Wqkv, bqkv, Wproj, bproj, trace=False)
    return out
